# revision 4
# baseline (speedup 1.0000x reference)
"""Trainium2 Bass kernel for PixelSNAIL-style strict-causal attention.

Problem: query/key/value [B=4, H=64, W=64, C=256] fp32.
  S = 4096 tokens per batch; scores = (Q K^T)/16 with strict causal mask
  (position i attends to j < i); out = softmax(scores) @ V (row 0 -> 0).

Strategy (8 NeuronCores):
  - 2 cores per batch: context-parallel split of the key/value blocks by
    parity (core h owns k-blocks h, h+2, ..., h+30). Every core runs the
    IDENTICAL program (SPMD) over all 32 query blocks of its batch.
  - No max-subtraction in softmax (scores ~ N(0,1), exp is safe in fp32),
    so per-core partial numerators/denominators combine exactly on host.
  - Host pre-transposes Q and K (c-major) so no on-chip transposes are
    needed; V gets a ones-column appended so the softmax denominator
    accumulates in PSUM alongside the numerator.
  - Scores matmuls run in float32r (full-rate fp32 PE mode).
  - The strict-causal/diagonal masking is data-driven: an additive mask
    input applied only on each q-slot's last position-pair.

Layout per core (b = core//2, h = core%2):
  qt_in [256, 4096]  = Q[b]^T
  kt_in [256, 2048]  = K[b][blocks h::2]^T
  v_in  [2048, 258]  = V[b][blocks h::2] ++ ones column
  m_in  [128, 1024]  = additive mask for the last position-pair of a slot
  o_out [4096, 258]  = partial (numerator ++ denominator) for this core

Program: 8 q-slots of 512 rows; slot p iterates position pairs t=0..p;
  each pair computes St[k=128, q=1024(2 positions x 512)] = Kt^T Qt in PSUM,
  exp via ScalarE into SBUF (f32r), masked on the last pair, then PV
  matmuls accumulate O[q=128, 257] per q-sub-block in PSUM.
"""

import os

import numpy as np

B = 4
S = 4096          # 64*64 tokens per batch
C = 256
NBLK = 32         # 128-row k blocks per batch
NPOS = 16         # k blocks per core (parity split)
NSLOT = 8         # q slots of 512 rows
SCALE = 1.0 / 16.0
NEG = -1.0e30

_CACHE = {}


def _build_nc():
    import concourse.bacc as bacc
    import concourse.mybir as mybir
    import concourse.tile as tile

    F32 = mybir.dt.float32
    F32R = mybir.dt.float32r

    nc = bacc.Bacc("TRN2", target_bir_lowering=False, debug=False, num_devices=8)
    qt_in = nc.dram_tensor("qt_in", [C, S], F32, kind="ExternalInput").ap()
    kt_in = nc.dram_tensor("kt_in", [C, NPOS * 128], F32, kind="ExternalInput").ap()
    v_in = nc.dram_tensor("v_in", [NPOS * 128, 258], F32, kind="ExternalInput").ap()
    m_in = nc.dram_tensor("m_in", [128, 1024], F32, kind="ExternalInput").ap()
    o_out = nc.dram_tensor("o_out", [S, 258], F32, kind="ExternalOutput").ap()

    with tile.TileContext(nc) as tc:
        with (
            tc.tile_pool(name="const", bufs=1) as const,
            tc.tile_pool(name="pt", bufs=3) as ptp,
            tc.tile_pool(name="osb", bufs=4) as osbp,
            tc.tile_pool(name="st", bufs=2, space="PSUM") as stp,
            tc.tile_pool(name="op", bufs=4, space="PSUM") as opp,
        ):
            qt = []
            kt = []
            for c in range(2):
                q_t = const.tile([128, S], F32R, tag=f"qt{c}")
                nc.sync.dma_start(q_t[:], qt_in[c * 128:(c + 1) * 128, :].bitcast(F32R))
                qt.append(q_t)
                k_t = const.tile([128, NPOS * 128], F32R, tag=f"kt{c}")
                nc.sync.dma_start(k_t[:], kt_in[c * 128:(c + 1) * 128, :].bitcast(F32R))
                kt.append(k_t)
            vsb = const.tile([128, NPOS * 258], F32R, tag="v")
            for t in range(NPOS):
                nc.sync.dma_start(
                    vsb[:, t * 258:(t + 1) * 258],
                    v_in[t * 128:(t + 1) * 128, :].bitcast(F32R),
                )
            mask = const.tile([128, 1024], F32, tag="m")
            nc.sync.dma_start(mask[:], m_in[:])

            for p in range(NSLOT):
                o_ps = [
                    opp.tile([128, 258], F32, tag="o", name=f"o_ps{p}_{qs}")
                    for qs in range(4)
                ]
                for t in range(p + 1):
                    st = stp.tile([128, 1024], F32, tag="st")
                    for jp in range(2):
                        pos = 2 * t + jp
                        for c in range(2):
                            nc.tensor.matmul(
                                st[:, jp * 512:(jp + 1) * 512],
                                lhsT=kt[c][:, pos * 128:(pos + 1) * 128],
                                rhs=qt[c][:, p * 512:(p + 1) * 512],
                                start=(c == 0),
                                stop=(c == 1),
                            )
                    if t == p:
                        nc.vector.tensor_tensor(
                            st[:], st[:], mask[:], mybir.AluOpType.add
                        )
                    pt = ptp.tile([128, 1024], F32R, tag="pt")
                    nc.scalar.activation(
                        pt[:], st[:], mybir.ActivationFunctionType.Exp, scale=SCALE
                    )
                    for jp in range(2):
                        for qs in range(4):
                            if t == p and jp == 1 and qs < 2:
                                continue
                            nc.tensor.matmul(
                                o_ps[qs][:],
                                lhsT=pt[:, jp * 512 + qs * 128: jp * 512 + (qs + 1) * 128],
                                rhs=vsb[:, (2 * t + jp) * 258:(2 * t + jp + 1) * 258],
                                start=(t == 0 and jp == 0),
                                stop=(t == p and jp == (0 if qs < 2 else 1)),
                            )
                for qs in range(4):
                    ob = osbp.tile([128, 258], F32, tag="ob")
                    nc.vector.tensor_copy(ob[:], o_ps[qs][:])
                    nc.sync.dma_start(
                        o_out[p * 512 + qs * 128: p * 512 + (qs + 1) * 128, :], ob[:]
                    )
    nc.compile()
    return nc


def _get_nc():
    if "nc" not in _CACHE:
        _CACHE["nc"] = _build_nc()
    return _CACHE["nc"]


def _make_masks():
    """Additive masks [128, 1024] for the last position-pair of each slot.

    Free-dim layout: (jp in {0,1}) x (qs in {0..3}) x 128. On the last pair t=p,
    position jp holds k-block 4p + 2*jp + h vs q-sub-block 4p + qs:
      block <  qblock -> fully allowed (0)
      block == qblock -> strict lower-triangular (allowed iff q_local > k_local)
      block >  qblock -> fully blocked (NEG)
    """
    k_loc = np.arange(128)[:, None]
    q_loc = np.arange(128)[None, :]
    strict = np.where(q_loc > k_loc, 0.0, NEG).astype(np.float32)
    zeros = np.zeros((128, 128), np.float32)
    blocked = np.full((128, 128), NEG, np.float32)
    masks = []
    for h in range(2):
        chunks = []
        for jp in range(2):
            rel = 2 * jp + h  # k-block offset relative to 4p
            for qs in range(4):
                if rel < qs:
                    chunks.append(zeros)
                elif rel == qs:
                    chunks.append(strict)
                else:
                    chunks.append(blocked)
        masks.append(np.concatenate(chunks, axis=1))
    return masks


def _run(query, key, value, trace=False, trace_cores=None):
    from concourse.bass_utils import run_bass_kernel_spmd

    query = np.ascontiguousarray(np.asarray(query, dtype=np.float32)).reshape(B, S, C)
    key = np.ascontiguousarray(np.asarray(key, dtype=np.float32)).reshape(B, S, C)
    value = np.ascontiguousarray(np.asarray(value, dtype=np.float32)).reshape(B, S, C)

    masks = _make_masks()
    pad = np.zeros((NPOS * 128, 2), np.float32)
    pad[:, 0] = 1.0
    in_maps = []
    for core in range(8):
        b, h = core // 2, core % 2
        k_sel = key[b].reshape(NBLK, 128, C)[h::2].reshape(NPOS * 128, C)
        v_sel = value[b].reshape(NBLK, 128, C)[h::2].reshape(NPOS * 128, C)
        in_maps.append(
            {
                "qt_in": np.ascontiguousarray(query[b].T),
                "kt_in": np.ascontiguousarray(k_sel.T),
                "v_in": np.ascontiguousarray(np.concatenate([v_sel, pad], axis=1)),
                "m_in": masks[h],
            }
        )

    nc = _get_nc()
    res = run_bass_kernel_spmd(
        nc,
        in_maps,
        list(range(8)),
        trace=trace,
        trace_cores=trace_cores,
    )

    out = np.empty((B, S, C), np.float32)
    for b in range(B):
        o0 = res.results[2 * b]["o_out"].astype(np.float64)
        o1 = res.results[2 * b + 1]["o_out"].astype(np.float64)
        num = o0[:, :C] + o1[:, :C]
        den = o0[:, C] + o1[:, C]
        den = np.where(den == 0.0, 1.0, den)
        out[b] = (num / den[:, None]).astype(np.float32)
    return out.reshape(B, 64, 64, C), res


def kernel(query, key, value):
    out, _ = _run(query, key, value, trace=False)
    return out


# revision 5
# speedup vs baseline: 1.0187x; 1.0187x over previous
"""Trainium2 Bass kernel for PixelSNAIL-style strict-causal attention.

Problem: query/key/value [B=4, H=64, W=64, C=256] fp32.
  S = 4096 tokens per batch; scores = (Q K^T)/16 with strict causal mask
  (position i attends to j < i); out = softmax(scores) @ V (row 0 -> 0).

Strategy (8 NeuronCores):
  - 2 cores per batch: context-parallel split of the key/value blocks by
    parity (core h owns k-blocks h, h+2, ..., h+30). Every core runs the
    IDENTICAL program (SPMD) over all 32 query blocks of its batch.
  - No max-subtraction in softmax (scores ~ N(0,1), exp is safe in fp32),
    so per-core partial numerators/denominators combine exactly on host.
  - Host pre-transposes Q and K (c-major) so no on-chip transposes are
    needed; V gets a ones-column appended so the softmax denominator
    accumulates in PSUM alongside the numerator.
  - Scores matmuls run in float32r (full-rate fp32 PE mode).
  - The strict-causal/diagonal masking is data-driven: an additive mask
    input applied only on each q-slot's last position-pair.

Layout per core (b = core//2, h = core%2):
  qt_in [256, 4096]  = Q[b]^T
  kt_in [256, 2048]  = K[b][blocks h::2]^T
  v_in  [2048, 258]  = V[b][blocks h::2] ++ ones column
  m_in  [128, 1024]  = additive mask for the last position-pair of a slot
  o_out [4096, 258]  = partial (numerator ++ denominator) for this core

Program: 8 q-slots of 512 rows; slot p iterates position pairs t=0..p;
  each pair computes St[k=128, q=1024(2 positions x 512)] = Kt^T Qt in PSUM,
  exp via ScalarE into SBUF (f32r), masked on the last pair, then PV
  matmuls accumulate O[q=128, 257] per q-sub-block in PSUM.
"""

import os

import numpy as np

B = 4
S = 4096          # 64*64 tokens per batch
C = 256
NBLK = 32         # 128-row k blocks per batch
NPOS = 16         # k blocks per core (parity split)
NSLOT = 8         # q slots of 512 rows
SCALE = 1.0 / 16.0
NEG = -1.0e30

_CACHE = {}


def _build_nc():
    import concourse.bacc as bacc
    import concourse.mybir as mybir
    import concourse.tile as tile

    F32 = mybir.dt.float32
    F32R = mybir.dt.float32r

    nc = bacc.Bacc("TRN2", target_bir_lowering=False, debug=False, num_devices=8)
    qt_in = nc.dram_tensor("qt_in", [C, S], F32, kind="ExternalInput").ap()
    kt_in = nc.dram_tensor("kt_in", [C, NPOS * 128], F32, kind="ExternalInput").ap()
    v_in = nc.dram_tensor("v_in", [NPOS * 128, 258], F32, kind="ExternalInput").ap()
    m_in = nc.dram_tensor("m_in", [128, 1024], F32, kind="ExternalInput").ap()
    o_out = nc.dram_tensor("o_out", [S, 258], F32, kind="ExternalOutput").ap()

    with tile.TileContext(nc) as tc:
        with (
            tc.tile_pool(name="const", bufs=1) as const,
            tc.tile_pool(name="pt", bufs=3) as ptp,
            tc.tile_pool(name="osb", bufs=4) as osbp,
            tc.tile_pool(name="st", bufs=2, space="PSUM") as stp,
            tc.tile_pool(name="op", bufs=4, space="PSUM") as opp,
        ):
            qt = [const.tile([128, S], F32R, tag=f"qt{c}", name=f"qt{c}") for c in range(2)]
            kt = [
                const.tile([128, NPOS * 128], F32R, tag=f"kt{c}", name=f"kt{c}")
                for c in range(2)
            ]
            vsb = const.tile([128, NPOS * 258], F32R, tag="v")
            mask = const.tile([128, 1024], F32, tag="m")

            # Chunked loads in slot-consumption order, alternating the two
            # HWDGE queues (sync + scalar) so slot 0's inputs land first and
            # compute overlaps the remaining loads.
            dma_engs = [nc.sync, nc.scalar]
            n_dma = 0

            def dma(dst, src):
                nonlocal n_dma
                dma_engs[n_dma % 2].dma_start(dst, src)
                n_dma += 1

            dma(mask[:], m_in[:])
            for p in range(NSLOT):
                for c in range(2):
                    dma(
                        qt[c][:, p * 512:(p + 1) * 512],
                        qt_in[c * 128:(c + 1) * 128, p * 512:(p + 1) * 512].bitcast(F32R),
                    )
                for c in range(2):
                    dma(
                        kt[c][:, p * 256:(p + 1) * 256],
                        kt_in[c * 128:(c + 1) * 128, p * 256:(p + 1) * 256].bitcast(F32R),
                    )
                for t in (2 * p, 2 * p + 1):
                    dma(
                        vsb[:, t * 258:(t + 1) * 258],
                        v_in[t * 128:(t + 1) * 128, :].bitcast(F32R),
                    )

            for p in range(NSLOT):
                o_ps = [
                    opp.tile([128, 258], F32, tag="o", name=f"o_ps{p}_{qs}")
                    for qs in range(4)
                ]
                for t in range(p + 1):
                    st = stp.tile([128, 1024], F32, tag="st")
                    for jp in range(2):
                        pos = 2 * t + jp
                        for c in range(2):
                            nc.tensor.matmul(
                                st[:, jp * 512:(jp + 1) * 512],
                                lhsT=kt[c][:, pos * 128:(pos + 1) * 128],
                                rhs=qt[c][:, p * 512:(p + 1) * 512],
                                start=(c == 0),
                                stop=(c == 1),
                            )
                    if t == p:
                        nc.vector.tensor_tensor(
                            st[:], st[:], mask[:], mybir.AluOpType.add
                        )
                    pt = ptp.tile([128, 1024], F32R, tag="pt")
                    nc.scalar.activation(
                        pt[:], st[:], mybir.ActivationFunctionType.Exp, scale=SCALE
                    )
                    for jp in range(2):
                        for qs in range(4):
                            if t == p and jp == 1 and qs < 2:
                                continue
                            nc.tensor.matmul(
                                o_ps[qs][:],
                                lhsT=pt[:, jp * 512 + qs * 128: jp * 512 + (qs + 1) * 128],
                                rhs=vsb[:, (2 * t + jp) * 258:(2 * t + jp + 1) * 258],
                                start=(t == 0 and jp == 0),
                                stop=(t == p and jp == (0 if qs < 2 else 1)),
                            )
                for qs in range(4):
                    ob = osbp.tile([128, 258], F32, tag="ob")
                    nc.vector.tensor_copy(ob[:], o_ps[qs][:])
                    nc.sync.dma_start(
                        o_out[p * 512 + qs * 128: p * 512 + (qs + 1) * 128, :], ob[:]
                    )
    nc.compile()
    return nc


def _get_nc():
    if "nc" not in _CACHE:
        _CACHE["nc"] = _build_nc()
    return _CACHE["nc"]


def _make_masks():
    """Additive masks [128, 1024] for the last position-pair of each slot.

    Free-dim layout: (jp in {0,1}) x (qs in {0..3}) x 128. On the last pair t=p,
    position jp holds k-block 4p + 2*jp + h vs q-sub-block 4p + qs:
      block <  qblock -> fully allowed (0)
      block == qblock -> strict lower-triangular (allowed iff q_local > k_local)
      block >  qblock -> fully blocked (NEG)
    """
    k_loc = np.arange(128)[:, None]
    q_loc = np.arange(128)[None, :]
    strict = np.where(q_loc > k_loc, 0.0, NEG).astype(np.float32)
    zeros = np.zeros((128, 128), np.float32)
    blocked = np.full((128, 128), NEG, np.float32)
    masks = []
    for h in range(2):
        chunks = []
        for jp in range(2):
            rel = 2 * jp + h  # k-block offset relative to 4p
            for qs in range(4):
                if rel < qs:
                    chunks.append(zeros)
                elif rel == qs:
                    chunks.append(strict)
                else:
                    chunks.append(blocked)
        masks.append(np.concatenate(chunks, axis=1))
    return masks


def _run(query, key, value, trace=False, trace_cores=None):
    from concourse.bass_utils import run_bass_kernel_spmd

    query = np.ascontiguousarray(np.asarray(query, dtype=np.float32)).reshape(B, S, C)
    key = np.ascontiguousarray(np.asarray(key, dtype=np.float32)).reshape(B, S, C)
    value = np.ascontiguousarray(np.asarray(value, dtype=np.float32)).reshape(B, S, C)

    masks = _make_masks()
    pad = np.zeros((NPOS * 128, 2), np.float32)
    pad[:, 0] = 1.0
    in_maps = []
    for core in range(8):
        b, h = core // 2, core % 2
        k_sel = key[b].reshape(NBLK, 128, C)[h::2].reshape(NPOS * 128, C)
        v_sel = value[b].reshape(NBLK, 128, C)[h::2].reshape(NPOS * 128, C)
        in_maps.append(
            {
                "qt_in": np.ascontiguousarray(query[b].T),
                "kt_in": np.ascontiguousarray(k_sel.T),
                "v_in": np.ascontiguousarray(np.concatenate([v_sel, pad], axis=1)),
                "m_in": masks[h],
            }
        )

    nc = _get_nc()
    res = run_bass_kernel_spmd(
        nc,
        in_maps,
        list(range(8)),
        trace=trace,
        trace_cores=trace_cores,
    )

    out = np.empty((B, S, C), np.float32)
    for b in range(B):
        o0 = res.results[2 * b]["o_out"].astype(np.float64)
        o1 = res.results[2 * b + 1]["o_out"].astype(np.float64)
        num = o0[:, :C] + o1[:, :C]
        den = o0[:, C] + o1[:, C]
        den = np.where(den == 0.0, 1.0, den)
        out[b] = (num / den[:, None]).astype(np.float32)
    return out.reshape(B, 64, 64, C), res


def kernel(query, key, value):
    out, _ = _run(query, key, value, trace=False)
    return out


# revision 7
# speedup vs baseline: 1.0447x; 1.0255x over previous
"""Trainium2 Bass kernel for PixelSNAIL-style strict-causal attention.

Problem: query/key/value [B=4, H=64, W=64, C=256] fp32.
  S = 4096 tokens per batch; scores = (Q K^T)/16 with strict causal mask
  (position i attends to j < i); out = softmax(scores) @ V (row 0 -> 0).

Strategy (8 NeuronCores):
  - 2 cores per batch: context-parallel split of the key/value blocks by
    parity (core h owns k-blocks h, h+2, ..., h+30). Every core runs the
    IDENTICAL program (SPMD) over all 32 query blocks of its batch.
  - No max-subtraction in softmax (scores ~ N(0,1), exp is safe in fp32),
    so per-core partial numerators/denominators combine exactly on host.
  - Host pre-transposes Q and K (c-major) so no on-chip transposes are
    needed; V gets a ones-column appended so the softmax denominator
    accumulates in PSUM alongside the numerator.
  - Scores matmuls run in float32r (full-rate fp32 PE mode).
  - The strict-causal/diagonal masking is data-driven: an additive mask
    input applied only on each q-slot's last position-pair.

Layout per core (b = core//2, h = core%2):
  qt_in [256, 4096]  = Q[b]^T
  kt_in [256, 2048]  = K[b][blocks h::2]^T
  v_in  [2048, 258]  = V[b][blocks h::2] ++ ones column
  m_in  [128, 768]  = additive mask for the last position-pair of a slot
  o_out [4096, 258]  = partial (numerator ++ denominator) for this core

Program: 8 q-slots of 512 rows; slot p iterates position pairs t=0..p;
  each pair computes St[k=128, q=1024(2 positions x 512)] = Kt^T Qt in PSUM,
  exp via ScalarE into SBUF (f32r), masked on the last pair, then PV
  matmuls accumulate O[q=128, 257] per q-sub-block in PSUM.
"""

import os

import numpy as np

B = 4
S = 4096          # 64*64 tokens per batch
C = 256
NBLK = 32         # 128-row k blocks per batch
NPOS = 16         # k blocks per core (parity split)
NSLOT = 8         # q slots of 512 rows
SCALE = 1.0 / 16.0
NEG = -1.0e30

_CACHE = {}


def _build_nc():
    import concourse.bacc as bacc
    import concourse.mybir as mybir
    import concourse.tile as tile

    F32 = mybir.dt.float32
    F32R = mybir.dt.float32r

    nc = bacc.Bacc("TRN2", target_bir_lowering=False, debug=False, num_devices=8)
    qt_in = nc.dram_tensor("qt_in", [C, S], F32, kind="ExternalInput").ap()
    kt_in = nc.dram_tensor("kt_in", [C, NPOS * 128], F32, kind="ExternalInput").ap()
    v_in = nc.dram_tensor("v_in", [NPOS * 128, 258], F32, kind="ExternalInput").ap()
    m_in = nc.dram_tensor("m_in", [128, 768], F32, kind="ExternalInput").ap()
    o_out = nc.dram_tensor("o_out", [S, 258], F32, kind="ExternalOutput").ap()

    with tile.TileContext(nc) as tc:
        with (
            tc.tile_pool(name="const", bufs=1) as const,
            tc.tile_pool(name="pt", bufs=3) as ptp,
            tc.tile_pool(name="osb", bufs=4) as osbp,
            tc.tile_pool(name="st", bufs=2, space="PSUM") as stp,
            tc.tile_pool(name="op", bufs=4, space="PSUM") as opp,
        ):
            qt = [const.tile([128, S], F32R, tag=f"qt{c}", name=f"qt{c}") for c in range(2)]
            kt = [
                const.tile([128, NPOS * 128], F32R, tag=f"kt{c}", name=f"kt{c}")
                for c in range(2)
            ]
            vsb = const.tile([128, NPOS * 258], F32R, tag="v")
            mask = const.tile([128, 768], F32, tag="m")

            # Chunked loads in slot-consumption order, alternating the two
            # HWDGE queues (sync + scalar) so slot 0's inputs land first and
            # compute overlaps the remaining loads.
            dma_engs = [nc.sync, nc.scalar]
            n_dma = 0

            def dma(dst, src):
                nonlocal n_dma
                dma_engs[n_dma % 2].dma_start(dst, src)
                n_dma += 1

            def dma_qt(p):
                for c in range(2):
                    dma(
                        qt[c][:, p * 512:(p + 1) * 512],
                        qt_in[c * 128:(c + 1) * 128, p * 512:(p + 1) * 512].bitcast(F32R),
                    )

            dma(mask[:], m_in[:])
            dma_qt(NSLOT - 1)
            for t in range(NSLOT):
                for c in range(2):
                    dma(
                        kt[c][:, t * 256:(t + 1) * 256],
                        kt_in[c * 128:(c + 1) * 128, t * 256:(t + 1) * 256].bitcast(F32R),
                    )
                for pos in (2 * t, 2 * t + 1):
                    dma(
                        vsb[:, pos * 258:(pos + 1) * 258],
                        v_in[pos * 128:(pos + 1) * 128, :].bitcast(F32R),
                    )
            for p in range(NSLOT - 2, -1, -1):
                dma_qt(p)

            for p in range(NSLOT - 1, -1, -1):
                o_ps = [
                    opp.tile([128, 258], F32, tag="o", name=f"o_ps{p}_{qs}")
                    for qs in range(4)
                ]
                for t in range(p + 1):
                    last = t == p
                    # On the last pair, position jp=1 is fully blocked for
                    # q-sub-blocks 0,1 on both cores: compute only the live
                    # 256-column half.
                    width = 768 if last else 1024
                    st = stp.tile([128, 1024], F32, tag="st", name=f"st{p}_{t}")
                    for jp in range(2):
                        pos = 2 * t + jp
                        qoff = p * 512 + (256 if (last and jp == 1) else 0)
                        n = 256 if (last and jp == 1) else 512
                        for c in range(2):
                            nc.tensor.matmul(
                                st[:, jp * 512:jp * 512 + n],
                                lhsT=kt[c][:, pos * 128:(pos + 1) * 128],
                                rhs=qt[c][:, qoff:qoff + n],
                                start=(c == 0),
                                stop=(c == 1),
                            )
                    if last:
                        nc.vector.tensor_tensor(
                            st[:, :width], st[:, :width], mask[:, :width],
                            mybir.AluOpType.add,
                        )
                    pt = ptp.tile([128, 1024], F32R, tag="pt", name=f"pt{p}_{t}")
                    nc.scalar.activation(
                        pt[:, :width], st[:, :width],
                        mybir.ActivationFunctionType.Exp, scale=SCALE,
                    )
                    for jp in range(2):
                        for qs in range(4):
                            if last and jp == 1 and qs < 2:
                                continue
                            loff = jp * 512 + qs * 128
                            if last and jp == 1:
                                loff = 512 + (qs - 2) * 128
                            nc.tensor.matmul(
                                o_ps[qs][:],
                                lhsT=pt[:, loff:loff + 128],
                                rhs=vsb[:, (2 * t + jp) * 258:(2 * t + jp + 1) * 258],
                                start=(t == 0 and jp == 0),
                                stop=(last and jp == (0 if qs < 2 else 1)),
                            )
                for qs in range(4):
                    ob = osbp.tile([128, 258], F32, tag="ob", name=f"ob{p}_{qs}")
                    nc.vector.tensor_copy(ob[:], o_ps[qs][:])
                    nc.sync.dma_start(
                        o_out[p * 512 + qs * 128: p * 512 + (qs + 1) * 128, :], ob[:]
                    )
    nc.compile()
    return nc


def _get_nc():
    if "nc" not in _CACHE:
        _CACHE["nc"] = _build_nc()
    return _CACHE["nc"]


def _make_masks():
    """Additive masks [128, 1024] for the last position-pair of each slot.

    Free-dim layout: (jp in {0,1}) x (qs in {0..3}) x 128. On the last pair t=p,
    position jp holds k-block 4p + 2*jp + h vs q-sub-block 4p + qs:
      block <  qblock -> fully allowed (0)
      block == qblock -> strict lower-triangular (allowed iff q_local > k_local)
      block >  qblock -> fully blocked (NEG)
    """
    k_loc = np.arange(128)[:, None]
    q_loc = np.arange(128)[None, :]
    strict = np.where(q_loc > k_loc, 0.0, NEG).astype(np.float32)
    zeros = np.zeros((128, 128), np.float32)
    blocked = np.full((128, 128), NEG, np.float32)
    masks = []
    for h in range(2):
        chunks = []
        for jp, qs_list in ((0, (0, 1, 2, 3)), (1, (2, 3))):
            rel = 2 * jp + h  # k-block offset relative to 4p
            for qs in qs_list:
                if rel < qs:
                    chunks.append(zeros)
                elif rel == qs:
                    chunks.append(strict)
                else:
                    chunks.append(blocked)
        masks.append(np.concatenate(chunks, axis=1))
    return masks


def _run(query, key, value, trace=False, trace_cores=None):
    from concourse.bass_utils import run_bass_kernel_spmd

    query = np.ascontiguousarray(np.asarray(query, dtype=np.float32)).reshape(B, S, C)
    key = np.ascontiguousarray(np.asarray(key, dtype=np.float32)).reshape(B, S, C)
    value = np.ascontiguousarray(np.asarray(value, dtype=np.float32)).reshape(B, S, C)

    masks = _make_masks()
    pad = np.zeros((NPOS * 128, 2), np.float32)
    pad[:, 0] = 1.0
    in_maps = []
    for core in range(8):
        b, h = core // 2, core % 2
        k_sel = key[b].reshape(NBLK, 128, C)[h::2].reshape(NPOS * 128, C)
        v_sel = value[b].reshape(NBLK, 128, C)[h::2].reshape(NPOS * 128, C)
        in_maps.append(
            {
                "qt_in": np.ascontiguousarray(query[b].T),
                "kt_in": np.ascontiguousarray(k_sel.T),
                "v_in": np.ascontiguousarray(np.concatenate([v_sel, pad], axis=1)),
                "m_in": masks[h],
            }
        )

    nc = _get_nc()
    res = run_bass_kernel_spmd(
        nc,
        in_maps,
        list(range(8)),
        trace=trace,
        trace_cores=trace_cores,
    )

    out = np.empty((B, S, C), np.float32)
    for b in range(B):
        o0 = res.results[2 * b]["o_out"].astype(np.float64)
        o1 = res.results[2 * b + 1]["o_out"].astype(np.float64)
        num = o0[:, :C] + o1[:, :C]
        den = o0[:, C] + o1[:, C]
        den = np.where(den == 0.0, 1.0, den)
        out[b] = (num / den[:, None]).astype(np.float32)
    return out.reshape(B, 64, 64, C), res


def kernel(query, key, value):
    out, _ = _run(query, key, value, trace=False)
    return out


# revision 8
# speedup vs baseline: 1.0682x; 1.0225x over previous
"""Trainium2 Bass kernel for PixelSNAIL-style strict-causal attention.

Problem: query/key/value [B=4, H=64, W=64, C=256] fp32.
  S = 4096 tokens per batch; scores = (Q K^T)/16 with strict causal mask
  (position i attends to j < i); out = softmax(scores) @ V (row 0 -> 0).

Strategy (8 NeuronCores):
  - 2 cores per batch: context-parallel split of the key/value blocks by
    parity (core h owns k-blocks h, h+2, ..., h+30). Every core runs the
    IDENTICAL program (SPMD) over all 32 query blocks of its batch.
  - No max-subtraction in softmax (scores ~ N(0,1), exp is safe in fp32),
    so per-core partial numerators/denominators combine exactly on host.
  - Host pre-transposes Q and K (c-major) so no on-chip transposes are
    needed; V gets a ones-column appended so the softmax denominator
    accumulates in PSUM alongside the numerator.
  - Scores matmuls run in float32r (full-rate fp32 PE mode).
  - The strict-causal/diagonal masking is data-driven: an additive mask
    input applied only on each q-slot's last position-pair.

Layout per core (b = core//2, h = core%2):
  qt_in [256, 4096]  = Q[b]^T
  kt_in [256, 2048]  = K[b][blocks h::2]^T
  v_in  [2048, 258]  = V[b][blocks h::2] ++ ones column
  m_in  [128, 768]  = additive mask for the last position-pair of a slot
  o_out [4096, 258]  = partial (numerator ++ denominator) for this core

Program: 8 q-slots of 512 rows; slot p iterates position pairs t=0..p;
  each pair computes St[k=128, q=1024(2 positions x 512)] = Kt^T Qt in PSUM,
  exp via ScalarE into SBUF (f32r), masked on the last pair, then PV
  matmuls accumulate O[q=128, 257] per q-sub-block in PSUM.
"""

import os

import numpy as np

B = 4
S = 4096          # 64*64 tokens per batch
C = 256
NBLK = 32         # 128-row k blocks per batch
NPOS = 16         # k blocks per core (parity split)
NSLOT = 8         # q slots of 512 rows
SCALE = 1.0 / 16.0
NEG = -1.0e30

_CACHE = {}


def _build_nc():
    import concourse.bacc as bacc
    import concourse.mybir as mybir
    import concourse.tile as tile

    F32 = mybir.dt.float32
    F32R = mybir.dt.float32r

    nc = bacc.Bacc("TRN2", target_bir_lowering=False, debug=False, num_devices=8)
    qt_in = nc.dram_tensor("qt_in", [C, S], F32, kind="ExternalInput").ap()
    kt_in = nc.dram_tensor("kt_in", [C, NPOS * 128], F32, kind="ExternalInput").ap()
    v_in = nc.dram_tensor("v_in", [NPOS * 128, 258], F32, kind="ExternalInput").ap()
    m_in = nc.dram_tensor("m_in", [128, 768], F32, kind="ExternalInput").ap()
    o_out = nc.dram_tensor("o_out", [S, 258], F32, kind="ExternalOutput").ap()

    with tile.TileContext(nc) as tc:
        with (
            tc.tile_pool(name="const", bufs=1) as const,
            tc.tile_pool(name="pt", bufs=3) as ptp,
            tc.tile_pool(name="osb", bufs=4) as osbp,
            tc.tile_pool(name="st", bufs=2, space="PSUM") as stp,
            tc.tile_pool(name="op", bufs=4, space="PSUM") as opp,
        ):
            qt = [const.tile([128, S], F32R, tag=f"qt{c}", name=f"qt{c}") for c in range(2)]
            kt = [
                const.tile([128, NPOS * 128], F32R, tag=f"kt{c}", name=f"kt{c}")
                for c in range(2)
            ]
            vsb = const.tile([128, NPOS * 258], F32R, tag="v")
            mask = const.tile([128, 768], F32, tag="m")

            # Chunked loads in slot-consumption order, alternating the two
            # HWDGE queues (sync + scalar) so slot 0's inputs land first and
            # compute overlaps the remaining loads.
            dma_engs = [nc.sync, nc.scalar]
            n_dma = 0

            def dma(dst, src):
                nonlocal n_dma
                dma_engs[n_dma % 2].dma_start(dst, src)
                n_dma += 1

            def dma_qt(p):
                for c in range(2):
                    dma(
                        qt[c][:, p * 512:(p + 1) * 512],
                        qt_in[c * 128:(c + 1) * 128, p * 512:(p + 1) * 512].bitcast(F32R),
                    )

            dma(mask[:], m_in[:])
            dma_qt(NSLOT - 1)
            for t in range(NSLOT):
                for c in range(2):
                    dma(
                        kt[c][:, t * 256:(t + 1) * 256],
                        kt_in[c * 128:(c + 1) * 128, t * 256:(t + 1) * 256].bitcast(F32R),
                    )
                # V rides a third queue (gpsimd SWDGE) in parallel
                for pos in (2 * t, 2 * t + 1):
                    nc.gpsimd.dma_start(
                        vsb[:, pos * 258:(pos + 1) * 258],
                        v_in[pos * 128:(pos + 1) * 128, :].bitcast(F32R),
                    )
            for p in range(NSLOT - 2, -1, -1):
                dma_qt(p)

            for p in range(NSLOT - 1, -1, -1):
                o_ps = [
                    opp.tile([128, 258], F32, tag="o", name=f"o_ps{p}_{qs}")
                    for qs in range(4)
                ]
                for t in range(p + 1):
                    last = t == p
                    # On the last pair, position jp=1 is fully blocked for
                    # q-sub-blocks 0,1 on both cores: compute only the live
                    # 256-column half.
                    width = 768 if last else 1024
                    st = stp.tile([128, 1024], F32, tag="st", name=f"st{p}_{t}")
                    for jp in range(2):
                        pos = 2 * t + jp
                        qoff = p * 512 + (256 if (last and jp == 1) else 0)
                        n = 256 if (last and jp == 1) else 512
                        for c in range(2):
                            nc.tensor.matmul(
                                st[:, jp * 512:jp * 512 + n],
                                lhsT=kt[c][:, pos * 128:(pos + 1) * 128],
                                rhs=qt[c][:, qoff:qoff + n],
                                start=(c == 0),
                                stop=(c == 1),
                            )
                    if last:
                        nc.vector.tensor_tensor(
                            st[:, :width], st[:, :width], mask[:, :width],
                            mybir.AluOpType.add,
                        )
                    pt = ptp.tile([128, 1024], F32R, tag="pt", name=f"pt{p}_{t}")
                    nc.scalar.activation(
                        pt[:, :width], st[:, :width],
                        mybir.ActivationFunctionType.Exp, scale=SCALE,
                    )
                    for jp in range(2):
                        for qs in range(4):
                            if last and jp == 1 and qs < 2:
                                continue
                            loff = jp * 512 + qs * 128
                            if last and jp == 1:
                                loff = 512 + (qs - 2) * 128
                            nc.tensor.matmul(
                                o_ps[qs][:],
                                lhsT=pt[:, loff:loff + 128],
                                rhs=vsb[:, (2 * t + jp) * 258:(2 * t + jp + 1) * 258],
                                start=(t == 0 and jp == 0),
                                stop=(last and jp == (0 if qs < 2 else 1)),
                            )
                for qs in range(4):
                    ob = osbp.tile([128, 258], F32, tag="ob", name=f"ob{p}_{qs}")
                    nc.vector.tensor_copy(ob[:], o_ps[qs][:])
                    nc.sync.dma_start(
                        o_out[p * 512 + qs * 128: p * 512 + (qs + 1) * 128, :], ob[:]
                    )
    nc.compile()
    return nc


def _get_nc():
    if "nc" not in _CACHE:
        _CACHE["nc"] = _build_nc()
    return _CACHE["nc"]


def _make_masks():
    """Additive masks [128, 1024] for the last position-pair of each slot.

    Free-dim layout: (jp in {0,1}) x (qs in {0..3}) x 128. On the last pair t=p,
    position jp holds k-block 4p + 2*jp + h vs q-sub-block 4p + qs:
      block <  qblock -> fully allowed (0)
      block == qblock -> strict lower-triangular (allowed iff q_local > k_local)
      block >  qblock -> fully blocked (NEG)
    """
    k_loc = np.arange(128)[:, None]
    q_loc = np.arange(128)[None, :]
    strict = np.where(q_loc > k_loc, 0.0, NEG).astype(np.float32)
    zeros = np.zeros((128, 128), np.float32)
    blocked = np.full((128, 128), NEG, np.float32)
    masks = []
    for h in range(2):
        chunks = []
        for jp, qs_list in ((0, (0, 1, 2, 3)), (1, (2, 3))):
            rel = 2 * jp + h  # k-block offset relative to 4p
            for qs in qs_list:
                if rel < qs:
                    chunks.append(zeros)
                elif rel == qs:
                    chunks.append(strict)
                else:
                    chunks.append(blocked)
        masks.append(np.concatenate(chunks, axis=1))
    return masks


def _run(query, key, value, trace=False, trace_cores=None):
    from concourse.bass_utils import run_bass_kernel_spmd

    query = np.ascontiguousarray(np.asarray(query, dtype=np.float32)).reshape(B, S, C)
    key = np.ascontiguousarray(np.asarray(key, dtype=np.float32)).reshape(B, S, C)
    value = np.ascontiguousarray(np.asarray(value, dtype=np.float32)).reshape(B, S, C)

    masks = _make_masks()
    pad = np.zeros((NPOS * 128, 2), np.float32)
    pad[:, 0] = 1.0
    in_maps = []
    for core in range(8):
        b, h = core // 2, core % 2
        k_sel = key[b].reshape(NBLK, 128, C)[h::2].reshape(NPOS * 128, C)
        v_sel = value[b].reshape(NBLK, 128, C)[h::2].reshape(NPOS * 128, C)
        in_maps.append(
            {
                "qt_in": np.ascontiguousarray(query[b].T),
                "kt_in": np.ascontiguousarray(k_sel.T),
                "v_in": np.ascontiguousarray(np.concatenate([v_sel, pad], axis=1)),
                "m_in": masks[h],
            }
        )

    nc = _get_nc()
    res = run_bass_kernel_spmd(
        nc,
        in_maps,
        list(range(8)),
        trace=trace,
        trace_cores=trace_cores,
    )

    out = np.empty((B, S, C), np.float32)
    for b in range(B):
        o0 = res.results[2 * b]["o_out"].astype(np.float64)
        o1 = res.results[2 * b + 1]["o_out"].astype(np.float64)
        num = o0[:, :C] + o1[:, :C]
        den = o0[:, C] + o1[:, C]
        den = np.where(den == 0.0, 1.0, den)
        out[b] = (num / den[:, None]).astype(np.float32)
    return out.reshape(B, 64, 64, C), res


def kernel(query, key, value):
    out, _ = _run(query, key, value, trace=False)
    return out
